# revision 24
# baseline (speedup 1.0000x reference)
"""BinaryMLP (dense_mlp) Trainium2 kernel — 8-core data-parallel sync-BN.

Strategy:
  - Shard batch (4096) across 8 NeuronCores (512 rows each); replicate weights.
  - Activations live in SBUF transposed: [features -> partitions, batch -> free].
    BatchNorm batch stats are then free-axis reductions (VectorE / ACT accum).
  - Matmuls: lhsT = W.T tile (stationary), rhs = xT tile (moving), fp32 PSUM
    accumulation. The two binary layers (weights exactly +-1) run in fp8e4
    with perf_mode=DoubleRow (2 k-slices per PE pass); a per-layer k-split
    lets part of the contraction stay bf16 to control quantization noise.
    Weight sign() / transposes / casts / slab packing done host-side.
  - Sync-BN: per-feature sum / sumsq reduced locally, then one 32KB fp32
    AllReduce per BN layer across the 8 cores.
  - Final Linear flips layout back to [batch -> partitions, classes -> free] by
    using the h3 activation tiles as the stationary operand; log_softmax is a
    free-axis max/exp-accum/ln chain; b3 is folded in via an extra
    ones-row x b3-row contraction tile.
"""

import os
import sys

for _p in ("/opt/trn_rl_repo",):
    if _p not in sys.path and os.path.isdir(_p):
        sys.path.insert(0, _p)

import numpy as np
import ml_dtypes

import concourse.bass as bass
import concourse.mybir as mybir
import concourse.tile as tile
from concourse import bacc
from concourse.bass_utils import run_bass_kernel_spmd

AF = mybir.ActivationFunctionType
ALU = mybir.AluOpType
F32 = mybir.dt.float32
BF16 = mybir.dt.bfloat16
F8 = mybir.dt.float8e4
DR = mybir.MatmulPerfMode.DoubleRow
AX = mybir.AxisListType

NP_BF16 = ml_dtypes.bfloat16
NP_F8 = ml_dtypes.float8_e4m3  # IEEE e4m3: max normal 240 == TRN fp8e4

P = 128
N_CORES = 8
B_TOTAL = 4096
D_IN = 4096
H1, H2, H3 = 4096, 4096, 2048
C = 1000
BN_EPS = 1e-5

KT0, MT0 = D_IN // P, H1 // P  # 32, 32
KT1, MT1 = H1 // P, H2 // P  # 32, 32
KT2, MT2 = H2 // P, H3 // P  # 32, 16
KT3 = H3 // P  # 16 (+1 aug tile for the bias)
MG = 4  # out-feature tiles per PSUM group (4 banks; 2 groups in flight)
KPAIR = 4  # k-tiles per weight-slab DMA

# fp8 config: number of k-tiles of each layer's contraction done in fp8
# DoubleRow (must be even, multiple of KPAIR for slab alignment). The fp8
# k-tiles of layer l+1 equal the fp8-written out-feature tiles of layer l.
KF1 = 32  # layer1 (binary) fp8 k-tiles (of KT1=32); h1 features < KF1*128 in fp8
KF2 = 32  # layer2 (binary) fp8 k-tiles (of KT2=32); h2 features < KF2*128 in fp8


def build(b_shard: int, n_cores: int):
    """Build + compile the SPMD program for a per-core batch shard of b_shard."""
    assert b_shard % P == 0
    nb = b_shard // P  # batch tiles for the final layer
    n_batch_global = b_shard * n_cores
    rg = [list(range(n_cores))]

    nc = bacc.Bacc(
        "TRN2", target_bir_lowering=False, debug=False, num_devices=n_cores
    )

    def wshape(kt, mt, dt):
        # packed slab layout: [p, kp_block, group, kk, cols]
        return [P, kt // KPAIR, mt // MG, KPAIR, MG * P]

    xT = nc.dram_tensor("xT", [D_IN, b_shard], BF16, kind="ExternalInput").ap()
    w0t = nc.dram_tensor("w0t", wshape(KT0, MT0, BF16), BF16, kind="ExternalInput").ap()
    w1t = nc.dram_tensor("w1t", wshape(KT1, MT1, F8), F8, kind="ExternalInput").ap()
    w2t = nc.dram_tensor("w2t", wshape(KT2, MT2, F8), F8, kind="ExternalInput").ap()
    # bf16 copies of the binary weights for the k >= KF region (absent if full fp8)
    w1b = (
        nc.dram_tensor("w1b", wshape(KT1, MT1, BF16), BF16, kind="ExternalInput").ap()
        if KF1 < KT1
        else None
    )
    w2b = (
        nc.dram_tensor("w2b", wshape(KT2, MT2, BF16), BF16, kind="ExternalInput").ap()
        if KF2 < KT2
        else None
    )
    # W3.T augmented with a b3 row (row H3) + zero padding to a full k-tile.
    w3t = nc.dram_tensor("w3t", [(KT3 + 1) * P, C], BF16, kind="ExternalInput").ap()
    g0p = nc.dram_tensor("g0p", [P, MT0], F32, kind="ExternalInput").ap()
    b0p = nc.dram_tensor("b0p", [P, MT0], F32, kind="ExternalInput").ap()
    g1p = nc.dram_tensor("g1p", [P, MT1], F32, kind="ExternalInput").ap()
    b1p = nc.dram_tensor("b1p", [P, MT1], F32, kind="ExternalInput").ap()
    g2p = nc.dram_tensor("g2p", [P, MT2], F32, kind="ExternalInput").ap()
    b2p = nc.dram_tensor("b2p", [P, MT2], F32, kind="ExternalInput").ap()
    out = nc.dram_tensor("out", [b_shard, C], F32, kind="ExternalOutput").ap()

    with tile.TileContext(nc) as tc:
        with (
            tc.tile_pool(name="big", bufs=1) as big,
            tc.tile_pool(name="wpool", bufs=3) as wpool,
            tc.tile_pool(name="psum", bufs=8, space="PSUM") as psum,
            tc.tile_pool(name="scratch", bufs=4) as scratch,
            tc.tile_pool(name="bn", bufs=8) as bnp,
            tc.tile_pool(name="small", bufs=24) as small,
            tc.tile_pool(name="dram", bufs=1, space="DRAM") as dram,
        ):
            # ---- persistent activation buffers -------------------------------
            # *_pre: bf16, holds pre-BN h (stats source copy); BN-apply writes
            # fp8 into *_f8 for features < KF*128, else in-place bf16 on _pre.
            xT_sb = big.tile([P, KT0, b_shard], BF16, name="xT_sb", tag="xT_sb")
            h1_pre = big.tile([P, MT0, b_shard], BF16, name="h1_pre", tag="h1_pre")
            h1_f8 = (
                big.tile([P, KF1, b_shard], F8, name="h1_f8", tag="h1_f8")
                if KF1 > 0
                else None
            )
            h2_pre = big.tile([P, MT1, b_shard], BF16, name="h2_pre", tag="h2_pre")
            h2_f8 = (
                big.tile([P, KF2, b_shard], F8, name="h2_f8", tag="h2_f8")
                if KF2 > 0
                else None
            )
            h3_sb = big.tile([P, MT2, b_shard], BF16, name="h3_sb", tag="h3_sb")
            ones_t = big.tile([P, b_shard], BF16, name="ones_t", tag="ones_t")

            nc.gpsimd.memset(ones_t[:], 0.0)
            nc.gpsimd.memset(ones_t[:1, :], 1.0)
            # xT loads in 512KB chunks, emitted just-in-time on the sync
            # queue interleaved with the weight-slab stream (one-chunk
            # lookahead) so the first matmuls start within a few us.
            xT_r = xT.rearrange("(ko p) b -> p ko b", p=P)
            XCH = 4  # k-tiles per xT chunk
            n_xch = KT0 // XCH
            xch_emitted = [False] * n_xch

            def xT_loader(kp):
                want = min((kp + KPAIR - 1) // XCH, n_xch - 1)
                for c in range(want + 1):
                    if not xch_emitted[c]:
                        xch_emitted[c] = True
                        # first chunks on HWDGE (fast ramp); the rest on the
                        # idle gpsimd queue so they don't contend with the
                        # weight-slab stream on sync. Chunk 0 in halves so
                        # the first matmul starts ~1.5us sooner.
                        q = nc.sync if c < 2 else nc.gpsimd
                        lo = c * XCH
                        hi = (c + 1) * XCH
                        if c == 0:
                            q.dma_start(xT_sb[:, lo : lo + 2, :], xT_r[:, lo : lo + 2, :])
                            q.dma_start(xT_sb[:, lo + 2 : hi, :], xT_r[:, lo + 2 : hi, :])
                        else:
                            q.dma_start(xT_sb[:, lo:hi, :], xT_r[:, lo:hi, :])

            # BN gamma/beta (host packed to [P, MT])
            gb = {}

            def load_gb(specs):
                for nm, ap_, mt in specs:
                    t = big.tile([P, mt], F32, name=f"{nm}_sb", tag=f"{nm}_sb")
                    nc.gpsimd.dma_start(t[:], ap_)
                    gb[nm] = t

            load_gb((("g0", g0p, MT0), ("b0", b0p, MT0)))

            # warm the ACT Exp/Ln LUTs now (ACT is idle) so the softmax tail
            # doesn't pay the 1.28us table load on its critical path
            warm = small.tile([P, 1], F32, name="warm", tag="sm")
            nc.gpsimd.memset(warm[:], 1.0)
            nc.scalar.activation(warm[:], warm[:], AF.Exp)
            nc.scalar.activation(warm[:], warm[:], AF.Ln)

            def mlp_layer(
                lidx, in_bf, in_f8, f8_kt, kt, mt, w_f8, w_bf, g_sb, b_sb,
                out_pre, out_f8, f8_mt,
                chunks, cover_bounds=None, in_loader=None,
            ):
                """out <- relu(bn(in.T @ W.T)), transposed layout.

                Contraction k-tiles < f8_kt read in_f8 with fp8 DoubleRow
                (2 k-tiles per PE pass); the rest read in_bf in bf16.
                BN-apply writes out_f8 for feature tiles < f8_mt, else
                in-place bf16 on out_pre.

                Pipelined sync-BN: per-feature stats are all-reduced in
                len(chunks) chunks (sizes in m-tiles, group-aligned). Each
                chunk's AR dispatches as soon as its last group's bn_stats
                land; trailing chunks are single groups so their small ARs
                start early and their features unlock the next layer's
                matmuls incrementally. cover_bounds gives the k-split points
                of THIS layer's groups 0,1 (the previous layer's chunk
                boundaries), so k-ranges normalized early are consumed first.
                """
                ngroups = mt // MG
                n_ch = len(chunks)
                assert sum(chunks) == mt and all(c % MG == 0 for c in chunks)
                ch_lo = [sum(chunks[:c]) for c in range(n_ch)]  # m-tile bounds
                g2c = {}
                for c in range(n_ch):
                    for g in range(ch_lo[c] // MG, (ch_lo[c] + chunks[c]) // MG):
                        g2c[g] = c
                kb = list(cover_bounds) if cover_bounds else [kt // 2]
                if kb[-1] != kt:
                    kb = kb + [kt]
                inv_n = 1.0 / float(n_batch_global)
                chunk_m = list(chunks)

                def ctiles(pool, free, nm):
                    return [
                        pool.tile(
                            [P] + free(chunk_m[c]), F32,
                            name=f"{nm}{lidx}_{c}", tag=f"{nm}{lidx}_{c}",
                        )
                        for c in range(n_ch)
                    ]

                stats = ctiles(big, lambda cm: [2 * cm], "stats")
                gstats = ctiles(big, lambda cm: [2 * cm], "gstats")
                arin = ctiles(dram, lambda cm: [2 * cm], "arin")
                arout = ctiles(dram, lambda cm: [2 * cm], "arout")
                s_sb = ctiles(big, lambda cm: [cm], "s")
                t_sb = ctiles(big, lambda cm: [cm], "t")
                # per-tile bn_stats 6-tuples: (n, mean, n*var) x (even, odd)
                st6 = ctiles(big, lambda cm: [cm, 6], "st6_")

                ps_tiles = {}

                def emit_mms(g, k_lo, k_hi):
                    if g not in ps_tiles:
                        ps_tiles[g] = [
                            psum.tile(
                                [P, b_shard], F32, name=f"ps{lidx}_{g}_{j}", tag="ps"
                            )
                            for j in range(MG)
                        ]
                    ps = ps_tiles[g]
                    for kp in range(k_lo, k_hi, KPAIR):
                        if in_loader is not None:
                            in_loader(kp)
                        use_f8 = kp < f8_kt
                        slab = wpool.tile(
                            [P, KPAIR, MG * P], F8 if use_f8 else BF16,
                            name=f"w{lidx}_{g}_{kp}", tag="wslab",
                        )
                        wsrc = (w_f8 if use_f8 else w_bf)[:, kp // KPAIR, g, :, :]
                        if in_loader is not None and kp == 0:
                            # halved first slabs: first matmuls start sooner
                            nc.sync.dma_start(slab[:, 0:2, :], wsrc[:, 0:2, :])
                            nc.sync.dma_start(slab[:, 2:KPAIR, :], wsrc[:, 2:KPAIR, :])
                        else:
                            nc.sync.dma_start(slab[:], wsrc)
                        if use_f8:
                            for kk in range(0, KPAIR, 2):
                                k = kp + kk
                                for j in range(MG):
                                    nc.tensor.matmul(
                                        ps[j][:],
                                        slab[:, kk : kk + 2, j * P : (j + 1) * P],
                                        in_f8[:, k : k + 2, :],
                                        start=(k == 0),
                                        stop=(k == kt - 2 and f8_kt == kt),
                                        perf_mode=DR,
                                    )
                        else:
                            for kk in range(KPAIR):
                                k = kp + kk
                                for j in range(MG):
                                    nc.tensor.matmul(
                                        ps[j][:],
                                        slab[:, kk, j * P : (j + 1) * P],
                                        in_bf[:, k, :],
                                        start=(k == 0),
                                        stop=(k == kt - 1),
                                    )

                def emit_stats(g):
                    # stats fully on DVE (bn_stats): the AR dispatch can then
                    # never be stuck behind gstats-gated apply work on ACT
                    c = g2c[g]
                    for j in range(MG):
                        m = g * MG + j
                        ml = m - ch_lo[c]
                        nc.scalar.activation(
                            out_pre[:, m, :], ps_tiles[g][j][:], AF.Copy
                        )
                        nc.vector.bn_stats(st6[c][:, ml, :], ps_tiles[g][j][:])

                def emit_chunk_math(c):
                    # stats[:, :cm] = sum = 256*(mean_e + mean_o)
                    # stats[:, cm:] = sumsq = nvar_e + nvar_o
                    #                         + 256*(mean_e^2 + mean_o^2)
                    st = stats[c]
                    cm = chunk_m[c]
                    half_n = float(b_shard // 2)
                    me = st6[c][:, :, 1]
                    mo = st6[c][:, :, 4]
                    ve = st6[c][:, :, 2]
                    vo = st6[c][:, :, 5]
                    t1 = bnp.tile([P, cm], F32, name=f"cm1_{lidx}_{c}", tag="bn")
                    t2 = bnp.tile([P, cm], F32, name=f"cm2_{lidx}_{c}", tag="bn")
                    t3 = bnp.tile([P, cm], F32, name=f"cm3_{lidx}_{c}", tag="bn")
                    nc.vector.tensor_add(t1[:], me, mo)
                    nc.vector.tensor_scalar_mul(st[:, :cm], t1[:], half_n)
                    nc.vector.tensor_mul(t2[:], me, me)
                    nc.vector.tensor_mul(t3[:], mo, mo)
                    nc.vector.tensor_add(t2[:], t2[:], t3[:])
                    nc.vector.tensor_scalar_mul(t2[:], t2[:], half_n)
                    nc.vector.tensor_add(t3[:], ve, vo)
                    nc.vector.tensor_add(st[:, cm : 2 * cm], t2[:], t3[:])

                def emit_ar(c):
                    nc.gpsimd.dma_start(arin[c][:], stats[c][:])
                    nc.gpsimd.collective_compute(
                        "AllReduce",
                        ALU.add,
                        replica_groups=rg,
                        ins=[arin[c].opt()],
                        outs=[arout[c].opt()],
                    )
                    nc.gpsimd.dma_start(gstats[c][:], arout[c][:])

                def emit_apply_st(c):
                    # s = g * rsqrt(var+eps); t = beta - mean*s
                    gs = gstats[c]
                    cm = chunk_m[c]
                    m0 = ch_lo[c]
                    mex = bnp.tile([P, 2 * cm], F32, name=f"mex{lidx}_{c}", tag="bn2")
                    m2 = bnp.tile([P, cm], F32, name=f"m2{lidx}_{c}", tag="bn")
                    var = bnp.tile([P, cm], F32, name=f"var{lidx}_{c}", tag="bn")
                    inv = bnp.tile([P, cm], F32, name=f"inv{lidx}_{c}", tag="bn")
                    rstd = bnp.tile([P, cm], F32, name=f"rstd{lidx}_{c}", tag="bn")
                    tmp = bnp.tile([P, cm], F32, name=f"tmp{lidx}_{c}", tag="bn")
                    nc.scalar.activation(mex[:], gs[:], AF.Copy, scale=inv_n)
                    mean = mex[:, :cm]
                    ex2 = mex[:, cm:]
                    nc.vector.tensor_mul(m2[:], mean[:], mean[:])
                    nc.vector.tensor_sub(var[:], ex2[:], m2[:])
                    nc.vector.tensor_scalar_add(var[:], var[:], BN_EPS)
                    nc.vector.reciprocal(inv[:], var[:])
                    nc.scalar.activation(rstd[:], inv[:], AF.Sqrt)
                    nc.vector.tensor_mul(
                        s_sb[c][:], rstd[:], g_sb[:, m0 : m0 + cm]
                    )
                    nc.vector.tensor_mul(tmp[:], mean[:], s_sb[c][:])
                    nc.vector.tensor_sub(
                        t_sb[c][:], b_sb[:, m0 : m0 + cm], tmp[:]
                    )

                def emit_apply(c, lo=0, hi=None):
                    # relu(h*s + t) for [lo, hi) of this chunk's feature tiles
                    cm = chunk_m[c]
                    m0 = ch_lo[c]
                    if hi is None:
                        hi = cm
                    for ml in range(lo, hi):
                        m = m0 + ml
                        dst = (
                            out_f8[:, m, :] if m < f8_mt else out_pre[:, m, :]
                        )
                        nc.scalar.activation(
                            dst,
                            out_pre[:, m, :],
                            AF.Relu,
                            bias=t_sb[c][:, ml : ml + 1],
                            scale=s_sb[c][:, ml : ml + 1],
                        )

                # groups 0,1: k-loop split at cover_bounds so each k-range
                # only needs the previous layer's already-applied chunks.
                # For the input layer, interleave in 4-k sub-blocks so the PE
                # can start as soon as the first 512KB input chunk lands.
                k_prev = 0
                for ci, k_hi in enumerate(kb):
                    if ci == 0 and in_loader is not None:
                        for k0 in range(k_prev, k_hi, 4):
                            emit_mms(0, k0, k0 + 4)
                            emit_mms(1, k0, k0 + 4)
                    else:
                        emit_mms(0, k_prev, k_hi)
                        emit_mms(1, k_prev, k_hi)
                    k_prev = k_hi
                emit_stats(0)
                emit_stats(1)
                # remaining groups of the first chunk
                for g in range(2, (ch_lo[0] + chunks[0]) // MG):
                    emit_mms(g, 0, kt)
                    emit_stats(g)
                emit_chunk_math(0)
                emit_ar(0)  # first-chunk AR overlaps the trailing groups
                # first-chunk applies emitted now: they only occupy ACT/DVE
                # ahead of later copies (non-critical); later chunks' bn_stats
                # on DVE can't be blocked by these gstats-gated ops, so every
                # AR dispatches as soon as its data is ready.
                emit_apply_st(0)
                emit_apply(0)
                for c in range(1, n_ch):
                    for g in range(ch_lo[c] // MG, (ch_lo[c] + chunks[c]) // MG):
                        emit_mms(g, 0, kt)
                        emit_stats(g)
                    emit_chunk_math(c)
                    emit_ar(c)
                # applies AFTER all trailing chunks' stats/ARs are emitted:
                # a late AR then can't wedge gstats-gated apply work ahead of
                # a later chunk's bn_stats in the DVE queue
                for c in range(1, n_ch):
                    emit_apply_st(c)
                    emit_apply(c)

            # 24-tile head chunk + two single-group tail chunks: the small
            # tail ARs dispatch ~14us apart so layer1's k>=24 ranges unlock
            # incrementally (the first collectives run slow, 23-32us, from
            # inter-core skew; the split hides most of it)
            mlp_layer(
                0, xT_sb, None, 0, KT0, MT0, None, w0t, gb["g0"], gb["b0"],
                h1_pre, h1_f8, KF1,
                chunks=(24, 4, 4), in_loader=xT_loader,
            )
            load_gb(
                (
                    ("g1", g1p, MT1),
                    ("b1", b1p, MT1),
                    ("g2", g2p, MT2),
                    ("b2", b2p, MT2),
                )
            )
            # 20-tile head + three single-group tails; cover splits at
            # layer0's chunk bounds (24, 28)
            mlp_layer(
                1, h1_pre, h1_f8, KF1, KT1, MT1, w1t, w1b, gb["g1"], gb["b1"],
                h2_pre, h2_f8, KF2,
                chunks=(20, 4, 4, 4), cover_bounds=[24, 28],
            )

            # preload ALL final-layer weight slabs now — the DMAs run during
            # layer 2's compute and layer 3 then never waits on weight loads
            # gpsimd (SWDGE) queue: runs during layer 2 without delaying the
            # sync-queue weight-slab stream
            w3_sb = big.tile([P, KT3 + 1, C], BF16, name="w3_sb", tag="w3_sb")
            nc.gpsimd.dma_start(
                w3_sb[:], w3t.rearrange("(ko p) c -> p ko c", p=P)
            )

            # asymmetric chunks (3+1 groups): the tiny tail AR is covered
            # by layer 3's first 12 k-tiles. cover splits at layer1's
            # chunk bounds (20, 24, 28).
            mlp_layer(
                2, h2_pre, h2_f8, KF2, KT2, MT2, w2t, w2b, gb["g2"], gb["b2"],
                h3_sb, None, 0,
                chunks=(12, 4), cover_bounds=[20, 24, 28],
            )

            # ---- final Linear + log_softmax ---------------------------------
            # lhsT = h3 tile slice (stationary), rhs = preloaded W3.T slab
            # (moving). Output flips to [batch -> partitions, classes -> free].
            # k-loop split: first 8 k-tiles (layer 2's chunk-a) for every
            # batch tile first, covering layer 2's second stats-AR.
            half = (C + 1) // 2  # 500
            ka = 12  # layer 2's chunk-a feature tiles
            ps3 = [
                [
                    psum.tile([P, 512], F32, name=f"ps3_{b}_{h}", tag="ps")
                    for h in range(2)
                ]
                for b in range(nb)
            ]

            def l3_mms(b, ks, stop_k, start_k=None):
                for k in ks:
                    lhsT = (
                        h3_sb[:, k, b * P : (b + 1) * P]
                        if k < KT3
                        else ones_t[:, b * P : (b + 1) * P]
                    )
                    for h in range(2):
                        nc.tensor.matmul(
                            ps3[b][h][:, : half],
                            lhsT,
                            w3_sb[:, k, h * half : (h + 1) * half],
                            start=(k == start_k),
                            stop=(k == stop_k),
                        )

            for b in range(nb):
                # bias (ones) k-tile first — it's ungated by layer 2's BN
                # applies, so it runs while layer 2's stats-ARs are in flight
                l3_mms(b, [KT3] + list(range(ka)), None, start_k=KT3)

            # log_softmax, stage-batched across batch tiles so the ACT LUT
            # (Exp / Ln) is loaded once per stage instead of per tile
            nmax = [None] * nb
            s0 = [None] * nb
            s1 = [None] * nb
            lse = [None] * nb
            shift = [None] * nb
            for b in range(nb):
                l3_mms(b, range(ka, KT3), KT3 - 1)
                p0 = ps3[b][0][:, :half]
                p1 = ps3[b][1][:, :half]
                m0 = small.tile([P, 1], F32, name=f"m0_{b}", tag="sm")
                m1 = small.tile([P, 1], F32, name=f"m1_{b}", tag="sm")
                nmax[b] = small.tile([P, 1], F32, name=f"nmax_{b}", tag="sm")
                nc.vector.tensor_reduce(m0[:], p0, axis=AX.X, op=ALU.max)
                nc.vector.tensor_reduce(m1[:], p1, axis=AX.X, op=ALU.max)
                nc.vector.tensor_max(m0[:], m0[:], m1[:])
                nc.vector.tensor_scalar_mul(nmax[b][:], m0[:], -1.0)
            # exp outputs are never read (only the accum sums are); dump them
            # all into one buffer so they don't cycle the o0/o1 writeback pool
            edump = big.tile([P, 512], F32, name="edump", tag="edump")
            for b in range(nb):
                s0[b] = small.tile([P, 1], F32, name=f"s0_{b}", tag="sm")
                s1[b] = small.tile([P, 1], F32, name=f"s1_{b}", tag="sm")
                nc.scalar.activation(
                    edump[:, :half], ps3[b][0][:, :half], AF.Exp,
                    bias=nmax[b][:], scale=1.0, accum_out=s0[b][:],
                )
                nc.scalar.activation(
                    edump[:, :half], ps3[b][1][:, :half], AF.Exp,
                    bias=nmax[b][:], scale=1.0, accum_out=s1[b][:],
                )
            for b in range(nb):
                ssum = small.tile([P, 1], F32, name=f"ssum_{b}", tag="sm")
                lse[b] = small.tile([P, 1], F32, name=f"lse_{b}", tag="sm")
                nc.vector.tensor_add(ssum[:], s0[b][:], s1[b][:])
                nc.scalar.activation(lse[b][:], ssum[:], AF.Ln)
            for b in range(nb):
                shift[b] = small.tile([P, 1], F32, name=f"shift_{b}", tag="sm")
                nc.vector.tensor_sub(shift[b][:], nmax[b][:], lse[b][:])
            for b in range(nb):
                # writeback split across ACT and DVE so the two halves of
                # each tile shift in parallel
                o0 = scratch.tile([P, 512], F32, name=f"o0_{b}", tag="sq")
                o1 = scratch.tile([P, 512], F32, name=f"o1_{b}", tag="sq")
                nc.scalar.activation(
                    o0[:, :half], ps3[b][0][:, :half], AF.Identity,
                    bias=shift[b][:], scale=1.0,
                )
                nc.vector.tensor_scalar_add(
                    o1[:, :half], ps3[b][1][:, :half], shift[b][:]
                )
                # halves on different DMA queues so the 8 stores drain in
                # parallel instead of serializing on sync
                nc.sync.dma_start(out[b * P : (b + 1) * P, :half], o0[:, :half])
                nc.gpsimd.dma_start(out[b * P : (b + 1) * P, half:C], o1[:, :half])

    nc.compile()
    return nc


def pack_slabs(wt, kt, mt, np_dtype):
    """[K, M] row-major -> [P, kpb, g, KPAIR, MG*P] so each (kpb, g) slab is
    2KB/4KB contiguous per partition in DRAM."""
    K, M = kt * P, mt * P
    assert wt.shape == (K, M)
    a = wt.reshape(kt // KPAIR, KPAIR, P, mt // MG, MG * P)
    a = a.transpose(2, 0, 3, 1, 4)  # [p, kpb, g, kk, cols]
    return np.ascontiguousarray(a).astype(np_dtype)


def prep_inputs(inputs, b_shard: int, n_cores: int):
    """Host-side prep: shard x, transpose/cast/pack weights, pack BN params."""
    x = np.ascontiguousarray(inputs["x"], dtype=np.float32)

    def sign_f32(w):
        return np.where(w >= 0, np.float32(1.0), np.float32(-1.0))

    w0T = inputs["W0"].astype(np.float32).T  # [D_IN, H1]
    w1T = sign_f32(np.asarray(inputs["Wb1"], dtype=np.float32)).T
    w2T = sign_f32(np.asarray(inputs["Wb2"], dtype=np.float32)).T
    w3t_aug = np.zeros(((KT3 + 1) * P, C), dtype=np.float32)
    w3t_aug[:H3] = inputs["W3"].astype(np.float32).T
    w3t_aug[H3] = inputs["b3"].astype(np.float32)

    def pack(v, mt):
        return np.ascontiguousarray(
            np.asarray(v, dtype=np.float32).reshape(mt, P).T
        )

    shared = {
        "w0t": pack_slabs(w0T, KT0, MT0, NP_BF16),
        "w1t": pack_slabs(w1T, KT1, MT1, NP_F8),
        "w2t": pack_slabs(w2T, KT2, MT2, NP_F8),
        "w3t": np.ascontiguousarray(w3t_aug).astype(NP_BF16),
        "g0p": pack(inputs["g0"], MT0),
        "b0p": pack(inputs["beta0"], MT0),
        "g1p": pack(inputs["g1"], MT1),
        "b1p": pack(inputs["beta1"], MT1),
        "g2p": pack(inputs["g2"], MT2),
        "b2p": pack(inputs["beta2"], MT2),
    }
    if KF1 < KT1:
        shared["w1b"] = pack_slabs(w1T, KT1, MT1, NP_BF16)
    if KF2 < KT2:
        shared["w2b"] = pack_slabs(w2T, KT2, MT2, NP_BF16)
    in_maps = []
    for i in range(n_cores):
        xs = x[i * b_shard : (i + 1) * b_shard]  # [b_shard, D_IN]
        m = dict(shared)
        m["xT"] = np.ascontiguousarray(xs.T).astype(NP_BF16)  # [D_IN, b_shard]
        in_maps.append(m)
    return in_maps


_CACHE = {}


def _get_compiled(b_shard: int, n_cores: int):
    key = (b_shard, n_cores)
    if key not in _CACHE:
        _CACHE[key] = build(b_shard, n_cores)
    return _CACHE[key]


def kernel(**inputs) -> np.ndarray:
    b_shard = B_TOTAL // N_CORES
    nc = _get_compiled(b_shard, N_CORES)
    in_maps = prep_inputs(inputs, b_shard, N_CORES)
    last_err = None
    for _attempt in range(3):
        try:
            res = run_bass_kernel_spmd(nc, in_maps, core_ids=list(range(N_CORES)))
            break
        except Exception as e:  # transient NRT device flakes recover on retry
            last_err = e
    else:
        raise last_err
    out = np.concatenate([r["out"] for r in res.results], axis=0)
    return out.astype(np.float32)


if __name__ == "__main__":
    data = np.load("/tmp/ref_data.npz")
    inputs = {k: data[k] for k in data.files if k != "expected"}
    expected = data["expected"]
    actual = kernel(**inputs)
    err = np.abs(actual - expected)
    print("max abs err:", err.max())
    print("absmax-rel:", err.max() / np.abs(expected).max())
